# revision 1
# baseline (speedup 1.0000x reference)
"""Distributed Trainium2 (Bass/Tile) kernel for single-head latent attention.

Reference computation (B=4, S=4096, D=1024, DL=64):
    qkv = x @ Wd + bd; q,k,v = split(qkv)
    logits = (q @ k^T) / sqrt(DL) / TEMP, key-masked
    out = softmax(logits) @ v @ Wu + bu

Sharding: data-parallel over (batch, seq-half) -> 8 shards of 2048 query rows.
Each core recomputes K/V for its batch's keys from x (no collectives).

Key tricks:
  - Host-side mask compaction: only unmasked rows (~2040 of 4096, capped at
    K_CAP=2176) are gathered as keys, cutting the S^2 attention work ~2x.
    Pad slots get exp-bias -1e30 -> zero weight.
  - Softmax without row-max: scaled logits are bounded (~±95), shifted by
    -40 in the exp bias, so exp/sums stay finite in fp32 and the flash
    accumulation over key chunks is plain PSUM accumulation.
  - PV matmul lhsT is [ones | v] [128, 65]: row 0 of the accumulator is Z,
    rows 1:65 are ctxU. After normalizing by broadcast(1/Z) row 0 becomes
    exactly 1.0, and the up-projection rhs [bu; Wu] folds in the bias.
  - V is transposed from the projection layout [dl, keys] to the PV layout
    [keys, dl] by 17 tiny PE transposes riding the schedule's PE slack.
  - dtypes: x/Wd/q/k/Wu fp16 (bf16's 8-bit mantissa fails: exp amplifies
    logit error to ~1.2e-2; fp16's 10 bits keep it ~2.5e-3), exp/v bf16
    (exp values overflow fp16), out f16.

Schedule (the kernel is ACT+PE bound; exp is 34 ACTIVATEs of [128,1024]):
  - Inputs land as ~1MB DMAs (small transfers are descriptor-bound): x is
    relaid host-side so each transfer is contiguous. All concurrently
    outstanding DMAs on a ring progress at EQUAL rates (SDMA packet
    round-robin), so transfers are released in gated groups (a DVE memset
    on the dst creates a WAW dep that throttles the queue): group 1 is
    exactly what pre-attention needs (consts, key-range 0, q pass-A).
  - A ~5us run of dummy matmuls on Wd bridges the group-1 DMA wall and
    trips the PE HAM clock-gate to 2.4 GHz before the projections start;
    after that, densely emitted real work keeps it there. Any PE idle
    >3.4us re-throttles to 1.2 GHz and can poison a whole pass (the
    cold->PE-bound->never-rewarms spiral), so the pass A->B seam emits
    pass B's first two MM1/exp pairs before pass A's trailing MM2s.
  - Attention pass A starts right behind kv range 0 (~20us); kv ranges
    1-4, the 17 V transposes, and the pass-B q-projection ride in pass A's
    PE slack, spread <=4 MMs per chunk to bound exp-stream gaps.
  - Pass A's ctx copy / normalization / up-projection tiles ride inside
    pass B's ACT-paced stream; the tail (up tiles 7-15) runs on all 8 free
    psum banks with psum evacuation split across ACT and DVE, plus a few
    filler matmuls through the epilogue's DVE window to hold the clock.
"""

import sys

if "/opt/trn_rl_repo" not in sys.path:
    sys.path.insert(0, "/opt/trn_rl_repo")

import numpy as np

from concourse import bacc, tile
from concourse import mybir
from concourse.masks import make_identity

F32 = mybir.dt.float32
F32R = mybir.dt.float32r
BF16 = mybir.dt.bfloat16
F16 = mybir.dt.float16

B, S, D, DL = 4, 4096, 1024, 64
N_CORES = 8
S_LOC = S // 2          # 2048 query rows per core
SR = 512
JC = 128                # key chunk
NJK = 17                # compacted key chunks
K_CAP = NJK * JC        # 2176 >= max unmasked keys per batch (~2076 @ +3σ
                        # above the Binomial(4096,1/2) mean of 2048)
QH = 1024               # logits/exp q-tile width (one attention pass)
VB = 80                 # v_aug block stride: [pad(15) | ones(1) | v(64)],
                        # v at +16 (32B-aligned for the XBAR), lhsT reads +15
SCALE = 1.25            # 1/sqrt(64)/0.1
LOGIT_SHIFT = -40.0
MASKED_BIAS = -1e30

# key ranges for the kv projection: 4x512 + 128
KV_RANGES = [(0, 512), (512, 512), (1024, 512), (1536, 512), (2048, 128)]

_CACHE = {}


def build_graph():
    """Core-agnostic Bacc graph; each core's inputs are pre-sliced host-side
    (local query half + compacted keys of its batch, in contiguous slabs)."""
    nc = bacc.Bacc("TRN2", target_bir_lowering=False, debug=False,
                   num_devices=N_CORES)

    # xq: [128, half(2) x slab(8) x 1024]; xk: [128, range-major 8*w blocks]
    xq_d = nc.dram_tensor("xq", [128, 2 * 8 * QH], F16, kind="ExternalInput").ap()
    xk_d = nc.dram_tensor("xk", [128, 8 * K_CAP], F16, kind="ExternalInput").ap()
    wd_d = nc.dram_tensor("Wd", [128, 8 * 192], F16, kind="ExternalInput").ap()
    wub_d = nc.dram_tensor("Wub", [DL + 1, D], F16, kind="ExternalInput").ap()
    bdq_d = nc.dram_tensor("bd_q", [64, 1], F32, kind="ExternalInput").ap()
    bdkv_d = nc.dram_tensor("bd_kv", [128, 1], F32, kind="ExternalInput").ap()
    mb_d = nc.dram_tensor("maskbias", [128, NJK], F32, kind="ExternalInput").ap()
    out_d = nc.dram_tensor("out", [S_LOC, D], F16, kind="ExternalOutput").ap()

    with tile.TileContext(nc) as tc, nc.allow_low_precision(
            reason="bf16/f16 tiles feed full-rate PE matmuls; ~10-bit "
                   "mantissas are far inside the 2e-2 error budget"):
        with (
            tc.tile_pool(name="consts", bufs=1) as consts,
            tc.tile_pool(name="acts", bufs=1) as acts,
            tc.tile_pool(name="ep", bufs=8) as ep,
            tc.tile_pool(name="ob", bufs=6) as ob,
        ):
            # ---- DMA plan --------------------------------------------------
            # sync FIFO is the priority order: small consts first (wd gates
            # the PE), then kv ranges 0-2 and q pass-A (projections chew each
            # 1MB transfer as it lands), then ranges 3-4 + wub.
            # scalar queue: act-table preload, q pass-B; exps after.
            wd_s = consts.tile([128, 8 * 192], F16)
            nc.sync.dma_start(out=wd_s[:], in_=wd_d[:])
            bdq_s = consts.tile([64, 1], F32)
            nc.sync.dma_start(out=bdq_s[:], in_=bdq_d[:])
            bdkv_s = consts.tile([128, 1], F32)
            nc.sync.dma_start(out=bdkv_s[:], in_=bdkv_d[:])
            mb_s = consts.tile([128, NJK], F32)
            nc.sync.dma_start(out=mb_s[:], in_=mb_d[:])
            # preload the exp ACT table set early so the ~2.7us table-load
            # stall doesn't hit the exp stream at attention start
            act_warm = consts.tile([128, NJK], F32)
            nc.scalar.activation(act_warm[:], mb_s[:],
                                 mybir.ActivationFunctionType.Exp)
            ones_colf = consts.tile([1, 128], F32)
            nc.vector.memset(ones_colf[:], 1.0)
            ones_col = consts.tile([1, 128], F32R)
            nc.vector.tensor_copy(ones_col[:], ones_colf[:])
            # identity at partitions 64:128 (vT rows live there), bf16 to
            # match the bf16 vT transposes
            ident2f = consts.tile([128, 64], F32)
            nc.vector.memset(ident2f[:], 0.0)
            make_identity(nc, ident2f[64:128, :], nomemset=True)
            ident2 = consts.tile([128, 64], BF16)
            nc.vector.tensor_copy(ident2[:], ident2f[:])
            gate_scr = consts.tile([128, 4], F32)

            xq_sb = acts.tile([128, 2 * 8 * QH], F16)
            xk_sb = acts.tile([128, 8 * K_CAP], F16)

            # All concurrently-outstanding DMAs on a ring progress at EQUAL
            # rates (SDMA packet round-robin), so transfers are released in
            # gated groups: a DVE memset on the first dst column creates a
            # WAW dep that throttles the sync queue until the gate fires.
            def xk_range_dma(r):
                c0, w = KV_RANGES[r]
                nc.sync.dma_start(out=xk_sb[:, 8 * c0:8 * (c0 + w)],
                                  in_=xk_d[:, 8 * c0:8 * (c0 + w)])

            def xk_gate(r):
                c0, w = KV_RANGES[r]
                nc.vector.memset(xk_sb[:, 8 * c0:8 * c0 + 1], 0.0)

            def xq_dma(h):
                sl = slice(h * 4096, (h + 1) * 4096)
                nc.sync.dma_start(out=xq_sb[:, sl], in_=xq_d[:, sl])

            def xq_gate(h):
                nc.vector.memset(xq_sb[:, h * 4096:h * 4096 + 1], 0.0)

            # group 1: everything the pre-attention projections need
            xk_range_dma(0)
            xq_dma(0)
            xq_dma(1)
            wub_s = consts.tile([DL + 1, D], F16)

            qT_s = acts.tile([64, S_LOC], F16)
            kT_s = acts.tile([64, K_CAP], F16)
            # vT (projection layout [dl, keys]) at partitions 64:128, bf16
            # so the XBAR DMA-transpose can lift chunks 8-16 off the PE
            vT_hi = acts.tile([128, K_CAP], BF16)
            # PV stationary per key chunk: col +15 = ones, cols +16:+80 = v
            v_aug = acts.tile([128, NJK * VB], BF16)
            nc.vector.memset(v_aug[:], 1.0)
            ctxu_s = acts.tile([DL + 1, S_LOC], F32R)
            rzb_s = acts.tile([DL + 1, S_LOC], F32)
            rzb_scr = acts.tile([DL + 1, S_LOC], F32)
            ctxn_s = acts.tile([DL + 1, S_LOC], F16)

            # PSUM budget is exactly 8 banks:
            #   pl 2x[128,1024]f32 = 4, pc 1x[65,1024]f32 = 2,
            #   pp 2x[128,512]f32 = 2 (pass A)  ->  po 2x[128,512] (pass B)
            with (
                tc.tile_pool(name="pl", bufs=2, space="PSUM") as pl,
                tc.tile_pool(name="pc", bufs=1, space="PSUM") as pc,
            ):
                # ---- helpers -----------------------------------------------
                def q_col(s2, k):
                    return (s2 // 2) * 8192 + k * QH + (s2 % 2) * SR

                def q_proj_mms(s2, ps):
                    for k in range(8):
                        nc.tensor.matmul(
                            ps[:], wd_s[:, k * 192:k * 192 + 64],
                            xq_sb[:, q_col(s2, k):q_col(s2, k) + SR],
                            start=(k == 0), stop=(k == 7))

                def q_bias(s2, ps):
                    nc.vector.tensor_scalar_add(
                        qT_s[:, s2 * SR:(s2 + 1) * SR], ps[:64, :], bdq_s[:])

                def kv_mms(r, ps, ks):
                    c0, w = KV_RANGES[r]
                    for k in ks:
                        nc.tensor.matmul(
                            ps[:, 0:w], wd_s[:, k * 192 + 64:(k + 1) * 192],
                            xk_sb[:, 8 * c0 + k * w:8 * c0 + (k + 1) * w],
                            start=(k == 0), stop=(k == 7))

                def kv_bias(r, ps):
                    c0, w = KV_RANGES[r]
                    nc.vector.tensor_scalar_add(kT_s[:, c0:c0 + w],
                                                ps[0:64, 0:w],
                                                bdkv_s[0:64, :])
                    nc.vector.tensor_scalar_add(vT_hi[64:128, c0:c0 + w],
                                                ps[64:128, 0:w],
                                                bdkv_s[64:128, :])

                # ---- pre-attention: q pass-A + kv ranges 0-2 ---------------
                with tc.tile_pool(name="pp", bufs=2, space="PSUM") as pp:
                    def v_transpose(c):
                        # [dl, keys] -> [keys, dl] on the PE (bf16 psum)
                        vt = pp.tile([128, 64], BF16, tag="p", name=f"vt{c}")
                        nc.tensor.transpose(
                            vt[:], vT_hi[64:128, c * JC:(c + 1) * JC],
                            ident2[64:128, :])
                        nc.vector.tensor_copy(
                            v_aug[:, c * VB + 16:c * VB + 80], vt[:])

                    def v_transpose_dma(c):
                        # chunks with late deadlines ride the sync queue's
                        # post-input idle time via the XBAR instead of the PE
                        nc.sync.dma_start(
                            out=v_aug[:, c * VB + 16:c * VB + 80],
                            in_=vT_hi[64:128, c * JC:(c + 1) * JC],
                            transpose=True)

                    def kv_range_full(r):
                        ps = pp.tile([128, SR], F32, tag="p", name=f"pskv{r}")
                        kv_mms(r, ps, range(8))
                        return ps

                    # HAM clock warmup: dummy matmuls on wd spanning the
                    # group-1 DMA wall so the projections (and everything
                    # after) run at 2.4 GHz instead of 1.2
                    warm_ps = pl.tile([128, SR], F32, tag="l", name="warm_ps")
                    for _ in range(12):
                        nc.tensor.matmul(warm_ps[:], wd_s[:, 0:128],
                                         wd_s[:, 0:SR], start=True, stop=True)
                    # gate: fires when the warmup (~group-1 completion) ends,
                    # releasing key-ranges 1-2 into the now-idle DMA ring
                    nc.vector.tensor_copy(gate_scr[:, 0:1], warm_ps[:, 0:1])
                    xk_gate(1)
                    xk_gate(2)
                    xk_range_dma(1)
                    xk_range_dma(2)
                    ps_kv0 = kv_range_full(0)
                    # k-bias first: it gates MM1(0) -> exp(0); v-bias can wait
                    nc.vector.tensor_scalar_add(kT_s[:, 0:SR],
                                                ps_kv0[0:64, 0:SR],
                                                bdkv_s[0:64, :])
                    ps_q0 = pl.tile([64, SR], F32, tag="l", name="ps_q0")
                    ps_q1 = pl.tile([64, SR], F32, tag="l", name="ps_q1")
                    for k in range(8):
                        nc.tensor.matmul(
                            ps_q0[:], wd_s[:, k * 192:k * 192 + 64],
                            xq_sb[:, q_col(0, k):q_col(0, k) + SR],
                            start=(k == 0), stop=(k == 7))
                        nc.tensor.matmul(
                            ps_q1[:], wd_s[:, k * 192:k * 192 + 64],
                            xq_sb[:, q_col(1, k):q_col(1, k) + SR],
                            start=(k == 0), stop=(k == 7))
                    q_bias(0, ps_q0)
                    q_bias(1, ps_q1)
                    nc.vector.tensor_scalar_add(vT_hi[64:128, 0:SR],
                                                ps_kv0[64:128, 0:SR],
                                                bdkv_s[64:128, :])

                    # ---- attention pass A with interleaved projections -----
                    kv_ps = {}

                    def kv_first(r):
                        def f():
                            kv_ps[r] = pp.tile([128, SR], F32, tag="p",
                                               name=f"pskv{r}")
                            kv_mms(r, kv_ps[r], range(4))
                        return f

                    def kv_second(r):
                        def f():
                            kv_mms(r, kv_ps[r], range(4, 8))
                            kv_bias(r, kv_ps[r])
                        return f

                    def kv_last():
                        def f():
                            kv_ps[4] = pp.tile([128, SR], F32, tag="p",
                                               name="pskv4")
                            kv_mms(4, kv_ps[4], range(8))
                            kv_bias(4, kv_ps[4])
                        return f

                    def vts(*cs):
                        def f():
                            for c in cs:
                                v_transpose(c)
                        return f

                    def vts_dma(*cs):
                        def f():
                            for c in cs:
                                v_transpose_dma(c)
                        return f

                    qb_ps = {}

                    def qproj_part(s2, k0, k1):
                        def f():
                            if k0 == 0:
                                qb_ps[s2] = pp.tile([64, SR], F32, tag="p",
                                                    name=f"psqb{s2}")
                            ps = qb_ps[s2]
                            for k in range(k0, k1):
                                nc.tensor.matmul(
                                    ps[:], wd_s[:, k * 192:k * 192 + 64],
                                    xq_sb[:, q_col(s2, k):q_col(s2, k) + SR],
                                    start=(k == 0), stop=(k == 7))
                        return f

                    def qbias_b(s2):
                        def f():
                            q_bias(s2, qb_ps[s2])
                        return f

                    def gate3():
                        xk_gate(3)
                        xk_gate(4)
                        xq_gate(2)
                        xk_range_dma(3)
                        xk_range_dma(4)
                        xq_dma(2)


                    def gate4():
                        xq_gate(3)
                        nc.vector.memset(wub_s[:, 0:1], 0.0)
                        xq_dma(3)
                        nc.sync.dma_start(out=wub_s[:], in_=wub_d[:])

                    extras = {
                        1: [vts(0, 1)],
                        2: [gate3, kv_first(1)],
                        3: [kv_second(1)],
                        4: [vts(2, 3)],
                        5: [gate4, kv_first(2)],
                        6: [kv_second(2)],
                        7: [vts(4, 5), vts_dma(8, 9, 10, 11)],
                        8: [vts(6), kv_first(3)],
                        9: [vts(7), kv_second(3)],
                        10: [vts_dma(12, 13, 14, 15), kv_last()],
                        11: [vts_dma(16), qproj_part(2, 0, 2)],
                        12: [qproj_part(2, 2, 4)],
                        13: [qproj_part(2, 4, 6)],
                        14: [qproj_part(2, 6, 8), qbias_b(2)],
                        15: [qproj_part(3, 0, 3)],
                        16: [qproj_part(3, 3, 6)],
                    }

                    ctx_tiles = {}
                    exs = {}

                    def mm1_exp(pas, c):
                        q0 = pas * QH
                        lg = pl.tile([128, QH], F32, tag="l",
                                     name=f"lg{pas}_{c}")
                        for s2 in range(2):
                            nc.tensor.matmul(
                                lg[:, s2 * SR:(s2 + 1) * SR],
                                kT_s[:, c * JC:(c + 1) * JC],
                                qT_s[:, q0 + s2 * SR:q0 + (s2 + 1) * SR],
                                start=True, stop=True)
                        ex = ep.tile([128, QH], BF16, tag="e",
                                     name=f"ex{pas}_{c}")
                        nc.scalar.activation(
                            ex[:], lg[:], mybir.ActivationFunctionType.Exp,
                            bias=mb_s[:, c:c + 1], scale=SCALE)
                        exs[(pas, c)] = ex

                    def mm2(pas, c):
                        ctx_ps = ctx_tiles[pas]
                        for s2 in range(2):
                            nc.tensor.matmul(
                                ctx_ps[:, s2 * SR:(s2 + 1) * SR],
                                v_aug[:, c * VB + 15:c * VB + 80],
                                exs[(pas, c)][:, s2 * SR:(s2 + 1) * SR],
                                start=(c == 0), stop=(c == NJK - 1))

                    # pass A
                    ctx_tiles[0] = pc.tile([DL + 1, QH], F32, tag="c",
                                           name="ctx0")
                    for c in range(NJK):
                        for f in extras.get(c, ()):
                            f()
                        mm1_exp(0, c)
                        if c >= 3:
                            mm2(0, c - 3)
                    # seam: keep the PE and ACT streams dense across the
                    # pass boundary (a >3.4us PE gap here re-throttles the
                    # clock and poisons all of pass B at 1.2 GHz)
                    qproj_part(3, 6, 8)()
                    qbias_b(3)()
                    mm1_exp(1, 0)
                    mm1_exp(1, 1)
                    for c in range(NJK - 3, NJK):
                        mm2(0, c)

                def up_tile(st, po, tail=False):
                    osb = ob.tile([128, D], F16, tag="ot", name=f"osb{st}")
                    for s2 in range(2):
                        up = po.tile([128, SR], F32, tag="o",
                                     name=f"up{st}_{s2}")
                        nc.tensor.matmul(
                            up[:], ctxn_s[:, st * 128:(st + 1) * 128],
                            wub_s[:, s2 * SR:(s2 + 1) * SR],
                            start=True, stop=True)
                        # tail: ACT is exp-free and DVE owns the epilogue, so
                        # ACT takes the larger share of psum evacuation
                        if tail and (s2 == 1 or st in (8, 12)):
                            nc.scalar.copy(osb[:, s2 * SR:(s2 + 1) * SR],
                                           up[:])
                        else:
                            nc.vector.tensor_copy(
                                osb[:, s2 * SR:(s2 + 1) * SR], up[:])
                    nc.sync.dma_start(out=out_d[st * 128:(st + 1) * 128, :],
                                      in_=osb[:])

                def epilogue_half(q0, po):
                    # normalize one 512-col slice: broadcast Z (ctx row 0)
                    # across all 65 rows, reciprocal, scale
                    sl = slice(q0, q0 + SR)
                    zb = po.tile([DL + 1, SR], F32, tag="o", name=f"zb{q0}")
                    nc.tensor.matmul(zb[:], ones_col[:, 0:DL + 1],
                                     ctxu_s[0:1, sl], start=True, stop=True)
                    nc.vector.reciprocal_approx_accurate(
                        rzb_s[:, sl], zb[:], rzb_scr[:, sl])
                    nc.vector.tensor_mul(ctxn_s[:, sl], ctxu_s[:, sl],
                                         rzb_s[:, sl])

                # pp closed -> 2 banks free for po (up-projection + Z bcast).
                # pass B; pass A's ctx copy / epilogue / up tiles ride in
                # the ACT-paced stream
                with tc.tile_pool(name="po", bufs=2, space="PSUM") as po:
                    for c in range(NJK):
                        if c == 2:
                            # ring-safe: ctx0's tail writes are all emitted
                            ctx_tiles[1] = pc.tile([DL + 1, QH], F32, tag="c",
                                                   name="ctx1")
                        if c >= 2:
                            mm1_exp(1, c)
                        if c == 1:
                            for s2 in range(2):
                                sl = slice(s2 * SR, (s2 + 1) * SR)
                                nc.vector.tensor_copy(
                                    ctxu_s[:, sl],
                                    ctx_tiles[0][:, s2 * SR:(s2 + 1) * SR])
                        if c == 2:
                            epilogue_half(0, po)
                        if c == 3:
                            epilogue_half(SR, po)
                        if c >= 4 and c % 2 == 0:
                            up_tile((c - 4) // 2, po)
                        if c >= 3:
                            mm2(1, c - 3)
                    for c in range(NJK - 3, NJK):
                        mm2(1, c)
                    nc.vector.tensor_copy(
                        ctxu_s[:, QH:QH + SR], ctx_tiles[1][:, 0:SR])
                    nc.scalar.copy(
                        ctxu_s[:, QH + SR:QH + 2 * SR],
                        ctx_tiles[1][:, SR:2 * SR])

            # pl/pc/po closed -> all 8 banks free for the dense tail: a
            # deep ring of wide 2-bank tiles so each tile needs only ONE
            # psum-evacuation copy, alternating ACT/DVE per tile
            with tc.tile_pool(name="pt", bufs=4, space="PSUM") as pt:
                up_tile(7, pt, tail=True)
                epilogue_half(QH, pt)
                epilogue_half(QH + SR, pt)
                for i in range(6):
                    fill = pt.tile([128, SR], F32, tag="o", name=f"fill{i}")
                    nc.tensor.matmul(fill[:], wd_s[:, 0:128], wd_s[:, 0:SR],
                                     start=True, stop=True)
                for st in range(8, 16):
                    osb = ob.tile([128, D], F16, tag="ot", name=f"osb{st}")
                    upw = pt.tile([128, D], F32, tag="o", name=f"upw{st}")
                    for s2 in range(2):
                        nc.tensor.matmul(
                            upw[:, s2 * SR:(s2 + 1) * SR],
                            ctxn_s[:, st * 128:(st + 1) * 128],
                            wub_s[:, s2 * SR:(s2 + 1) * SR],
                            start=True, stop=True)
                    if st % 2 == 0:
                        nc.scalar.copy(osb[:], upw[:])
                    else:
                        nc.vector.tensor_copy(osb[:], upw[:])
                    nc.sync.dma_start(out=out_d[st * 128:(st + 1) * 128, :],
                                      in_=osb[:])

    nc.compile()
    return nc


def get_graph():
    if "graph" not in _CACHE:
        _CACHE["graph"] = build_graph()
    return _CACHE["graph"]


def make_in_maps(x, attention_mask, Wd, bd, Wu, bu):
    # up-proj rhs [bu; Wu]: bias row first (ctx row 0 is the Z/ones row)
    wub = np.ascontiguousarray(
        np.concatenate([bu[None, :], Wu], axis=0).astype(np.float16))
    wd_c = np.ascontiguousarray(
        Wd.astype(np.float16).reshape(8, 128, 192).transpose(1, 0, 2)
        .reshape(128, 8 * 192))
    bd_q = np.ascontiguousarray(bd[0:64].reshape(64, 1).astype(np.float32))
    bd_kv = np.ascontiguousarray(bd[64:192].reshape(128, 1).astype(np.float32))
    per_batch = []
    for b in range(B):
        idx = np.nonzero(attention_mask[b])[0]
        n = len(idx)
        assert n <= K_CAP, f"unmasked key count {n} exceeds K_CAP={K_CAP}"
        idxp = np.concatenate([idx, np.zeros(K_CAP - n, np.int64)])
        # [8, 128, K_CAP] d-slabs -> range-major [128, 8*w] blocks so each
        # key-range is one contiguous ~1MB DMA
        xkT = x[b][idxp].T.astype(np.float16).reshape(8, 128, K_CAP)
        xk = np.concatenate(
            [xkT[:, :, c0:c0 + w].transpose(1, 0, 2).reshape(128, 8 * w)
             for c0, w in KV_RANGES], axis=1)
        mb = np.full(K_CAP, MASKED_BIAS, np.float32)
        mb[:n] = LOGIT_SHIFT
        per_batch.append((np.ascontiguousarray(xk),
                          np.ascontiguousarray(mb.reshape(NJK, 128).T)))
    in_maps = []
    for c in range(N_CORES):
        b, h = c // 2, c % 2
        xk, mb = per_batch[b]
        # [8, 128, S_LOC] d-slabs -> half-major [128, 2 x 8 x 1024] so each
        # attention pass's q input is two contiguous 1MB DMAs
        xT = x[b, h * S_LOC:(h + 1) * S_LOC].T.astype(np.float16) \
            .reshape(8, 128, S_LOC)
        xq = np.concatenate(
            [xT[:, :, hh * QH:(hh + 1) * QH].transpose(1, 0, 2)
             .reshape(128, 8 * QH) for hh in range(2)], axis=1)
        in_maps.append({
            "xq": np.ascontiguousarray(xq),
            "xk": xk,
            "Wd": wd_c,
            "Wub": wub,
            "bd_q": bd_q,
            "bd_kv": bd_kv,
            "maskbias": mb,
        })
    return in_maps


def kernel(x, attention_mask, Wd, bd, Wu, bu):
    from concourse import bass_utils

    x = np.asarray(x, dtype=np.float32)
    attention_mask = np.asarray(attention_mask)
    Wd = np.asarray(Wd, dtype=np.float32)
    bd = np.asarray(bd, dtype=np.float32)
    Wu = np.asarray(Wu, dtype=np.float32)
    bu = np.asarray(bu, dtype=np.float32)

    nc = get_graph()
    in_maps = make_in_maps(x, attention_mask, Wd, bd, Wu, bu)
    res = bass_utils.run_bass_kernel_spmd(nc, in_maps, list(range(N_CORES)))
    out = np.empty((B, S, D), dtype=np.float32)
    for c in range(N_CORES):
        b, h = c // 2, c % 2
        out[b, h * S_LOC:(h + 1) * S_LOC, :] = \
            res.results[c]["out"].astype(np.float32)
    return out

